# revision 1
# baseline (speedup 1.0000x reference)
"""Trainium2 Bass kernel for nn_AllObsPredictAtten (moe_routing).

Data-parallel over 8 NeuronCores: batch 8192 -> 1024 rows/core.
Per core, activations are kept feature-major ("transposed" layout:
[features on partitions, batch on free]) so every contraction runs on the
PE array with weights stationary.

 - x is DMA-loaded with an SWDGE f32->bf16 cast, transposed on the PE
   (bf16, 1 cyc/row), layer-1 matmuls run in bf16 with col-tiling
   (4 grid cells concurrently in the 128x128 array) and a block-diagonal
   one-hot matmul (4 cells per M=128 matmul).
 - Downstream layers (grid_comb, inv2, modules, attention) run in
   float32r (1 cyc/row at N=512, ~1e-4 accuracy).
 - All weight transforms (transposes, chunking, block-diagonals, bias
   stacking) are precomputed on the host and shipped as extra DRAM
   inputs (<1 MB, replicated to all cores).
 - softmax normalization is folded into the selection weights before the
   output-layer matmuls, so the final PSUM holds the finished output.

kernel(**inputs) caches the compiled 8-core program across calls.
"""
import sys

sys.path.insert(0, "/opt/trn_rl_repo")

import numpy as np
import ml_dtypes

import concourse.bacc as bacc
import concourse.bass as bass
import concourse.tile as tile
from concourse import mybir, bass2jax

F32 = mybir.dt.float32
F32R = mybir.dt.float32r
BF16 = mybir.dt.bfloat16

P = 128
BL = 1024           # batch rows per core
NCORES = 8
NSUP = BL // 512    # supertiles per core (N=512 each)
HID = 32

# x column map
GRID0 = 0            # 25 cells x 300
OH0 = 7500           # 25 cells x 7
GOAL0 = 7675         # 300
INV0 = 7975          # 10 cells x 300
XW = 10975

_CACHE = {}


# ----------------------------------------------------------------------------
# host-side parameter prep
# ----------------------------------------------------------------------------

def _prep_params(i):
    bf = ml_dtypes.bfloat16
    f32 = np.float32
    p = {}
    p["ident_bf"] = np.eye(P, dtype=bf)
    p["ident_f32"] = np.eye(P, dtype=f32)

    def chunkT(W, dt):  # W [32, F] -> [128, nk, 32] transposed chunks
        F = W.shape[1]
        nk = (F + 127) // 128
        out = np.zeros((P, nk, 32), dtype=dt)
        for k in range(nk):
            sz = min(128, F - 128 * k)
            out[:sz, k, :] = W[:, 128 * k:128 * k + sz].T.astype(dt)
        return out

    p["wg"] = chunkT(i["W_embed"], bf)        # [128, 3, 32] bf16
    p["winv1"] = chunkT(i["W_inv1"], bf)
    p["wgoal"] = chunkT(i["W_goal"], bf)
    p["wcomb"] = chunkT(i["W_comb"], f32)     # [128, 7, 32] f32r
    p["winv2"] = chunkT(i["W_inv2"], f32)     # [128, 3, 32]

    # block-diag onehot weights: quad q (cells 4q..4q+3, q6 = cell 24)
    # rows = A-local (7c, c<16) for q<=3, B-local (7c-112) for q>=4
    woh = np.zeros((P, 7, P), dtype=bf)
    WohT = i["W_onehot"].T  # [7, 32]
    for q in range(7):
        cells = range(4 * q, min(4 * q + 4, 25))
        for ci, c in enumerate(cells):
            r = 7 * c if c < 16 else 7 * c - 112
            woh[r:r + 7, q, 32 * ci:32 * ci + 32] = WohT.astype(bf)
    p["woh_bd"] = woh

    # in-layer grouped: [32i, 3j, 128(4a x 32o)]
    win = np.zeros((32, 3, P), dtype=f32)
    for j in range(3):
        for a in range(4):
            win[:, j, 32 * a:32 * a + 32] = i["in_W"][4 * j + a].T
    p["w_in_grp"] = win

    # out-layer stacked big-K: [128, 3, 32]: rows 32a+i = out_W[4j+a, o, i]
    wout = np.zeros((P, 3, 32), dtype=f32)
    for j in range(3):
        for a in range(4):
            wout[32 * a:32 * a + 32, j, :] = i["out_W"][4 * j + a].T
    p["w_out_big"] = wout

    p["w_att"] = i["att_W"].T.astype(f32).copy()      # [32, 12]
    E3 = np.zeros((12, 3, P), dtype=f32)
    for j in range(3):
        for a in range(4):
            E3[4 * j + a, j, 32 * a:32 * a + 32] = 1.0
    p["E3"] = E3
    p["ones12"] = np.ones((12, 1), dtype=f32)
    p["ones112"] = np.ones((1, 12), dtype=f32)
    p["outb"] = i["out_b"].astype(f32).copy()          # [12, 32]

    p["b_c1x4"] = np.tile(i["b_embed"], 4)[:, None].astype(f32)
    p["b_ohx4"] = np.tile(i["b_onehot"], 4)[:, None].astype(f32)
    p["b_i1x4"] = np.tile(i["b_inv1"], 4)[:, None].astype(f32)
    p["b_comb"] = i["b_comb"][:, None].astype(f32).copy()
    p["b_inv2"] = i["b_inv2"][:, None].astype(f32).copy()
    p["b_goal"] = i["b_goal"][:, None].astype(f32).copy()
    p["in_b_stk"] = i["in_b"].reshape(3, 128).T.astype(f32).copy()  # [128, 3]
    p["att_b"] = i["att_b"][:, None].astype(f32).copy()
    return p


_PARAM_DECLS = {
    "ident_bf": ([P, P], BF16),
    "ident_f32": ([P, P], F32),
    "wg": ([P, 3, 32], BF16),
    "winv1": ([P, 3, 32], BF16),
    "wgoal": ([P, 3, 32], BF16),
    "wcomb": ([P, 7, 32], F32R),
    "winv2": ([P, 3, 32], F32R),
    "woh_bd": ([P, 7, P], BF16),
    "w_in_grp": ([32, 3, P], F32R),
    "w_out_big": ([P, 3, 32], F32R),
    "w_att": ([32, 12], F32R),
    "E3": ([12, 3, P], F32R),
    "ones12": ([12, 1], F32R),
    "ones112": ([1, 12], F32R),
    "outb": ([12, 32], F32R),
    "b_c1x4": ([P, 1], F32),
    "b_ohx4": ([P, 1], F32),
    "b_i1x4": ([P, 1], F32),
    "b_comb": ([32, 1], F32),
    "b_inv2": ([32, 1], F32),
    "b_goal": ([32, 1], F32),
    "in_b_stk": ([P, 3], F32),
    "att_b": ([12, 1], F32),
}


# ----------------------------------------------------------------------------
# device program
# ----------------------------------------------------------------------------

def _build_nc(bench_r=0):
    import os
    KVAR = int(os.environ.get("KVAR", "4"))
    T_BUFS = int(os.environ.get("T_BUFS", "3"))
    C1_BUFS = int(os.environ.get("C1_BUFS", "1"))
    IO_BUFS = int(os.environ.get("IO_BUFS", "2"))
    WK_BUFS = int(os.environ.get("WK_BUFS", "2"))
    EV_PAT = os.environ.get("EV_PAT", "ad")  # rotation: a=ACT d=DVE
    nc = bacc.Bacc(None, target_bir_lowering=False)
    X = nc.declare_dram_parameter("x_s", [BL, XW], F32, isOutput=False)
    H = nc.declare_dram_parameter("h_s", [BL, HID], F32, isOutput=False)
    prm = {}
    for name, (shape, dt) in _PARAM_DECLS.items():
        prm[name] = nc.declare_dram_parameter(name, shape, dt, isOutput=False)
    OUT = nc.declare_dram_parameter("out", [BL, HID], F32, isOutput=True)

    with tile.TileContext(nc) as tc:
        with (
            tc.tile_pool(name="const", bufs=1) as cp,
            tc.tile_pool(name="io", bufs=IO_BUFS) as io,
            tc.tile_pool(name="work", bufs=WK_BUFS) as wk,
            tc.tile_pool(name="psT", bufs=T_BUFS, space="PSUM") as psT,
            tc.tile_pool(name="ps", bufs=1, space="PSUM") as ps,
            tc.tile_pool(name="psC", bufs=C1_BUFS, space="PSUM") as psC,
        ):
            # ---- constants to SBUF
            c = {}
            for name, (shape, dt) in _PARAM_DECLS.items():
                t = cp.tile(shape, dt, tag=name)
                nc.sync.dma_start(out=t[:], in_=prm[name].ap())
                c[name] = t

            evict_ctr = [0]
            import contextlib
            loop_ctx = tc.For_i(0, bench_r, 1) if bench_r > 1 else contextlib.nullcontext()


            def evict(out_ap, in_ap):
                # rotate PSUM->SBUF eviction engine per EV_PAT
                e = EV_PAT[evict_ctr[0] % len(EV_PAT)]
                if e == "a":
                    nc.scalar.copy(out_ap, in_ap)
                else:
                    nc.vector.tensor_copy(out_ap, in_ap)
                evict_ctr[0] += 1

          # bench_r>1 wraps the whole per-core program in a hardware loop
          # (same compute each iteration; results overwritten) for timing.
            with loop_ctx:
              for s in range(NSUP):
                R0 = 512 * s
                # ======== og (onehot+goal) + hiddens ========
                og = io.tile([P, 4, 475], BF16, tag="og")
                nc.gpsimd.dma_start(
                    out=og[:],
                    in_=X.ap()[R0:R0 + 512, OH0:OH0 + 475]
                    .rearrange("(bt p) f -> p bt f", p=P),
                )
                hid = io.tile([P, 4, HID], F32, tag="hid")
                nc.sync.dma_start(
                    out=hid[:],
                    in_=H.ap()[R0:R0 + 512, :]
                    .rearrange("(bt p) f -> p bt f", p=P),
                )

                ohTA = wk.tile([112, 512], BF16, tag="ohTA")
                ohTB = wk.tile([63, 512], BF16, tag="ohTB")
                goalT = wk.tile([P, 3, 512], BF16, tag="goalT")
                hidT = wk.tile([32, 512], F32R, tag="hidT")

                for bt in range(4):
                    cols = slice(128 * bt, 128 * bt + 128)
                    bA = psT.tile([P, 4, P], BF16, tag="T")
                    nc.tensor.transpose(bA[0:112, 0, :], og[:, bt, 0:112], c["ident_bf"][:])
                    nc.tensor.transpose(bA[0:63, 1, :], og[:, bt, 112:175], c["ident_bf"][:])
                    nc.tensor.transpose(bA[:, 2, :], og[:, bt, 175:303], c["ident_bf"][:])
                    nc.tensor.transpose(bA[:, 3, :], og[:, bt, 303:431], c["ident_bf"][:])
                    bB = psT.tile([P, 4, P], BF16, tag="T")
                    nc.tensor.transpose(bB[0:44, 0, :], og[:, bt, 431:475], c["ident_bf"][:])
                    bH = psT.tile([32, P], F32, tag="T")
                    nc.tensor.transpose(bH[:], hid[:, bt, :], c["ident_f32"][:])
                    evict(ohTA[:, cols], bA[0:112, 0, :])
                    evict(ohTB[:, cols], bA[0:63, 1, :])
                    evict(goalT[:, 0:2, cols], bA[:, 2:4, :])
                    evict(goalT[0:44, 2, cols], bB[0:44, 0, :])
                    nc.scalar.copy(hidT[:, cols], bH[:])

                # ---- goal embedding
                gps = ps.tile([32, 512], F32, tag="misc")
                for k, Kk in enumerate((128, 128, 44)):
                    nc.tensor.matmul(
                        gps[:], c["wgoal"][0:Kk, k, :], goalT[0:Kk, k, :],
                        start=(k == 0), stop=(k == 2),
                    )
                goal_emb = wk.tile([32, 512], F32R, tag="goal_emb")
                nc.scalar.activation(goal_emb[:], gps[:],
                                     mybir.ActivationFunctionType.Relu,
                                     bias=c["b_goal"][:])

                # ---- attention -> normalized selection
                aps = ps.tile([12, 512], F32, tag="misc")
                nc.tensor.matmul(aps[:], c["w_att"][:], hidT[:], start=True, stop=True)
                expsel_f = wk.tile([12, 512], F32, tag="expsel_f")
                nc.scalar.activation(expsel_f[:], aps[:],
                                     mybir.ActivationFunctionType.Exp,
                                     bias=c["att_b"][:])
                expsel_r = wk.tile([12, 512], F32R, tag="expsel_r")
                nc.scalar.copy(expsel_r[:], expsel_f[:])
                sps = ps.tile([1, 512], F32, tag="misc")
                nc.tensor.matmul(sps[:], c["ones12"][:], expsel_r[:], start=True, stop=True)
                recip = wk.tile([1, 512], F32, tag="recip")
                nc.vector.reciprocal(recip[:], sps[:])
                recip_r = wk.tile([1, 512], F32R, tag="recip_r")
                nc.scalar.copy(recip_r[:], recip[:])
                rps = ps.tile([12, 512], F32, tag="misc")
                nc.tensor.matmul(rps[:], c["ones112"][:], recip_r[:], start=True, stop=True)
                seln_f = wk.tile([12, 512], F32, tag="seln_f")
                nc.vector.tensor_mul(seln_f[:], expsel_f[:], rps[:])
                sel_n = wk.tile([12, 512], F32R, tag="sel_n")
                nc.scalar.copy(sel_n[:], seln_f[:])

                # ======== grid quads -> c1/c2 -> comb accumulation ========
                combps = ps.tile([32, 512], F32, tag="comb")
                for q in range(7 if KVAR >= 2 else 0):
                    ncell = 4 if q < 6 else 1
                    W = 300 * ncell
                    Wl = W + 84  # widened so 44-col transposes read 128 cols
                    xq = io.tile([P, 4, 1284], BF16, tag="xq")
                    nc.gpsimd.dma_start(
                        out=xq[:, :, 0:Wl],
                        in_=X.ap()[R0:R0 + 512, 1200 * q:1200 * q + Wl]
                        .rearrange("(bt p) f -> p bt f", p=P),
                    )
                    xT = wk.tile([P, 12, 512], BF16, tag="xT")
                    for bt in range(4):
                        cols = slice(128 * bt, 128 * bt + 128)
                        nslot = 3 * ncell
                        banks = []
                        for b in range((nslot + 3) // 4):
                            bk = psT.tile([P, 4, P], BF16, tag="T")
                            banks.append(bk)
                        for ci in range(ncell):
                            for k, off in enumerate((0, 128, 256)):
                                slot = 3 * ci + k
                                bk = banks[slot // 4]
                                nc.tensor.transpose(
                                    bk[:, slot % 4, :],
                                    xq[:, bt, 300 * ci + off:300 * ci + off + 128],
                                    c["ident_bf"][:],
                                )
                        for b, bk in enumerate(banks):
                            w = min(4, nslot - 4 * b)
                            evict(xT[:, 4 * b:4 * b + w, cols], bk[:, 0:w, :])
                    # c1: grid cell matmuls (col-tiled)
                    c1ps = psC.tile([P, 512], F32, tag="c1")
                    for ci in range(ncell):
                        for k, Kk in enumerate((128, 128, 44)):
                            nc.tensor.matmul(
                                c1ps[32 * ci:32 * ci + 32, :],
                                c["wg"][0:Kk, k, :],
                                xT[0:Kk, 3 * ci + k, :],
                                start=(k == 0), stop=(k == 2),
                                tile_position=(0, 32 * ci),
                            )
                    # c2: block-diag onehot matmul
                    c2ps = ps.tile([P, 512], F32, tag="c2")
                    Mq = 32 * ncell
                    if q <= 3:
                        nc.tensor.matmul(c2ps[0:Mq, :], c["woh_bd"][0:112, q, 0:Mq],
                                         ohTA[:, :], start=True, stop=True)
                    else:
                        nc.tensor.matmul(c2ps[0:Mq, :], c["woh_bd"][0:63, q, 0:Mq],
                                         ohTB[:, :], start=True, stop=True)
                    c1r = wk.tile([P, 512], F32R, tag="c1r")
                    c2r = wk.tile([P, 512], F32R, tag="c2r")
                    nc.scalar.activation(c1r[0:Mq, :], c1ps[0:Mq, :],
                                         mybir.ActivationFunctionType.Relu,
                                         bias=c["b_c1x4"][0:Mq, :])
                    nc.scalar.activation(c2r[0:Mq, :], c2ps[0:Mq, :],
                                         mybir.ActivationFunctionType.Relu,
                                         bias=c["b_ohx4"][0:Mq, :])
                    Kq = 128 if q < 6 else 32
                    nc.tensor.matmul(combps[:], c["wcomb"][0:Kq, q, :], c1r[0:Kq, :],
                                     start=(q == 0), stop=False)
                    nc.tensor.matmul(combps[:], c["wcomb"][0:Kq, q, :], c2r[0:Kq, :],
                                     start=False, stop=(q == 6))

                # ======== inventory quads -> inv2 accumulation ========
                inv2ps = ps.tile([32, 512], F32, tag="inv2")
                for iq in range(3 if KVAR >= 3 else 0):
                    ncell = (4, 4, 2)[iq]
                    W = 300 * ncell
                    xq = io.tile([P, 4, 1284], BF16, tag="xq")
                    nc.gpsimd.dma_start(
                        out=xq[:, :, 0:W + (84 if iq < 2 else 0)],
                        in_=X.ap()[R0:R0 + 512,
                                   INV0 + 1200 * iq:INV0 + 1200 * iq + W + (84 if iq < 2 else 0)]
                        .rearrange("(bt p) f -> p bt f", p=P),
                    )
                    if iq == 2:
                        # x ends here; pad the widened-transpose overread with
                        # arbitrary (unused) data so all PSUM partitions get written
                        nc.gpsimd.dma_start(
                            out=xq[:, :, W:W + 84],
                            in_=X.ap()[R0:R0 + 512, 0:84]
                            .rearrange("(bt p) f -> p bt f", p=P),
                        )
                    xT = wk.tile([P, 12, 512], BF16, tag="xT")
                    for bt in range(4):
                        cols = slice(128 * bt, 128 * bt + 128)
                        nslot = 3 * ncell
                        banks = []
                        for b in range((nslot + 3) // 4):
                            bk = psT.tile([P, 4, P], BF16, tag="T")
                            banks.append(bk)
                        for ci in range(ncell):
                            for k, off in enumerate((0, 128, 256)):
                                slot = 3 * ci + k
                                bk = banks[slot // 4]
                                nc.tensor.transpose(
                                    bk[:, slot % 4, :],
                                    xq[:, bt, 300 * ci + off:300 * ci + off + 128],
                                    c["ident_bf"][:],
                                )
                        for b, bk in enumerate(banks):
                            w = min(4, nslot - 4 * b)
                            evict(xT[:, 4 * b:4 * b + w, cols], bk[:, 0:w, :])
                    i1ps = psC.tile([P, 512], F32, tag="c1")
                    for ci in range(ncell):
                        for k, Kk in enumerate((128, 128, 44)):
                            nc.tensor.matmul(
                                i1ps[32 * ci:32 * ci + 32, :],
                                c["winv1"][0:Kk, k, :],
                                xT[0:Kk, 3 * ci + k, :],
                                start=(k == 0), stop=(k == 2),
                                tile_position=(0, 32 * ci),
                            )
                    Mq = 32 * ncell
                    invr = wk.tile([P, 512], F32R, tag="invr")
                    nc.scalar.activation(invr[0:Mq, :], i1ps[0:Mq, :],
                                         mybir.ActivationFunctionType.Relu,
                                         bias=c["b_i1x4"][0:Mq, :])
                    Kiq = (128, 128, 64)[iq]
                    nc.tensor.matmul(inv2ps[:], c["winv2"][0:Kiq, iq, :], invr[0:Kiq, :],
                                     start=(iq == 0), stop=(iq == 2))

                # ======== net embeddings ========
                if KVAR < 2:
                    nc.tensor.matmul(combps[0:12, :], c["w_att"][:], hidT[:], start=True, stop=True)
                if KVAR < 3:
                    nc.tensor.matmul(inv2ps[0:12, :], c["w_att"][:], hidT[:], start=True, stop=True)
                grid_comb = wk.tile([32, 512], F32R, tag="grid_comb")
                nc.scalar.activation(grid_comb[:], combps[:],
                                     mybir.ActivationFunctionType.Relu,
                                     bias=c["b_comb"][:])
                inv_emb = wk.tile([32, 512], F32R, tag="inv_emb")
                nc.scalar.activation(inv_emb[:], inv2ps[:],
                                     mybir.ActivationFunctionType.Relu,
                                     bias=c["b_inv2"][:])

                # ======== modules + weighted output ========
                outps = ps.tile([32, 512], F32, tag="misc")
                srcs = (grid_comb, inv_emb, goal_emb)
                if KVAR < 4:
                    nc.tensor.matmul(outps[0:12, :], c["w_att"][:], hidT[:], start=True, stop=True)
                for j in range(3 if KVAR >= 4 else 0):
                    hps = psC.tile([P, 512], F32, tag="c1")
                    nc.tensor.matmul(hps[:], c["w_in_grp"][:, j, :], srcs[j][:],
                                     start=True, stop=True)
                    hj = wk.tile([P, 512], F32, tag="hj")
                    nc.scalar.activation(hj[:], hps[:],
                                         mybir.ActivationFunctionType.Tanh,
                                         bias=c["in_b_stk"][:, j:j + 1])
                    Bps = ps.tile([P, 512], F32, tag="c2")
                    nc.tensor.matmul(Bps[:], c["E3"][:, j, :], sel_n[:],
                                     start=True, stop=True)
                    gf = wk.tile([P, 512], F32, tag="gf")
                    nc.vector.tensor_mul(gf[:], hj[:], Bps[:])
                    gr = wk.tile([P, 512], F32R, tag="gr")
                    nc.scalar.copy(gr[:], gf[:])
                    nc.tensor.matmul(
                        outps[:], c["w_out_big"][:, j, :], gr[:],
                        start=(j == 0), stop=False,
                    )
                if KVAR >= 4:
                    nc.tensor.matmul(outps[:], c["outb"][:], sel_n[:],
                                     start=False, stop=True)

                # ======== transpose back + store ========
                out_sb = wk.tile([32, 512], F32, tag="out_sb")
                nc.scalar.copy(out_sb[:], outps[:])
                out_nat = io.tile([P, 4, HID], F32, tag="out_nat")
                for bt in range(4):
                    tf = psT.tile([P, HID], F32, tag="T")
                    nc.tensor.transpose(tf[:], out_sb[:, 128 * bt:128 * bt + 128],
                                        c["ident_f32"][0:32, 0:32])
                    nc.vector.tensor_copy(out_nat[:, bt, :], tf[:])
                nc.sync.dma_start(
                    out=OUT.ap()[R0:R0 + 512, :].rearrange("(bt p) f -> p bt f", p=P),
                    in_=out_nat[:],
                )

    nc.finalize()
    return nc


# ----------------------------------------------------------------------------
# 8-core runner (jit once, reuse)
# ----------------------------------------------------------------------------

def _make_runner(nc):
    import jax
    from jax.sharding import Mesh, PartitionSpec
    from jax.experimental.shard_map import shard_map

    bass2jax.install_neuronx_cc_hook()
    partition_name = nc.partition_id_tensor.name if nc.partition_id_tensor else None
    in_names, out_names, out_avals = [], [], []
    for alloc in nc.m.functions[0].allocations:
        if not isinstance(alloc, mybir.MemoryLocationSet):
            continue
        name = alloc.memorylocations[0].name
        if alloc.kind == "ExternalInput":
            if name != partition_name:
                in_names.append(name)
        elif alloc.kind == "ExternalOutput":
            out_names.append(name)
            out_avals.append(jax.core.ShapedArray(
                tuple(alloc.tensor_shape), mybir.dt.np(alloc.dtype)))
    n_params = len(in_names)
    n_outs = len(out_avals)
    in_names_full = in_names + out_names
    if partition_name is not None:
        in_names_full = in_names_full + [partition_name]
    donate = tuple(range(n_params, n_params + n_outs))

    def _body(*args):
        operands = list(args)
        if partition_name is not None:
            operands.append(bass2jax.partition_id_tensor())
        outs = bass2jax._bass_exec_p.bind(
            *operands,
            out_avals=tuple(out_avals),
            in_names=tuple(in_names_full),
            out_names=tuple(out_names),
            lowering_input_output_aliases=(),
            sim_require_finite=True,
            sim_require_nnan=True,
            nc=nc,
        )
        return tuple(outs)

    devices = jax.devices()[:NCORES]
    mesh = Mesh(np.asarray(devices), ("core",))
    in_specs = (PartitionSpec("core"),) * (n_params + n_outs)
    out_specs = (PartitionSpec("core"),) * n_outs
    sharded = jax.jit(
        shard_map(_body, mesh=mesh, in_specs=in_specs, out_specs=out_specs,
                  check_rep=False),
        donate_argnums=donate, keep_unused=True,
    )

    _CACHE["sharded"] = sharded
    _CACHE["body"] = _body
    _CACHE["mesh"] = mesh
    _CACHE["in_names"] = in_names
    _CACHE["out_names"] = out_names
    _CACHE["out_avals"] = out_avals
    _CACHE["n_params"] = n_params

    def run(global_ins):
        # global_ins: dict name -> np array with leading dim NCORES*per_core
        ins = [global_ins[name] for name in in_names]
        zeros = [np.zeros((NCORES * a.shape[0], *a.shape[1:]), a.dtype)
                 for a in out_avals]
        outs = sharded(*ins, *zeros)
        import jax as _j
        _j.block_until_ready(outs)
        return {name: np.asarray(outs[i]) for i, name in enumerate(out_names)}

    return run


def _get_runner():
    if "runner" not in _CACHE:
        nc = _build_nc()
        _CACHE["runner"] = _make_runner(nc)
    return _CACHE["runner"]


def kernel(**inputs):
    run = _get_runner()
    prm = _prep_params(inputs)
    global_ins = {
        "x_s": np.ascontiguousarray(inputs["x"], dtype=np.float32),
        "h_s": np.ascontiguousarray(inputs["hiddens"], dtype=np.float32),
    }
    for name in _PARAM_DECLS:
        a = prm[name]
        global_ins[name] = np.concatenate([a] * NCORES, axis=0)
    outs = run(global_ins)
    return outs["out"]  # [8192, 32] f32


if __name__ == "__main__":
    rng = np.random.default_rng(0)
    fake = {
        "x": rng.standard_normal((8192, XW), dtype=np.float32),
        "hiddens": rng.standard_normal((8192, HID), dtype=np.float32),
        "W_embed": rng.standard_normal((32, 300), dtype=np.float32) * 0.05,
        "b_embed": rng.standard_normal((32,), dtype=np.float32) * 0.05,
        "W_onehot": rng.standard_normal((32, 7), dtype=np.float32) * 0.05,
        "b_onehot": rng.standard_normal((32,), dtype=np.float32) * 0.05,
        "W_comb": rng.standard_normal((32, 800), dtype=np.float32) * 0.05,
        "b_comb": rng.standard_normal((32,), dtype=np.float32) * 0.05,
        "W_inv1": rng.standard_normal((32, 300), dtype=np.float32) * 0.05,
        "b_inv1": rng.standard_normal((32,), dtype=np.float32) * 0.05,
        "W_inv2": rng.standard_normal((32, 320), dtype=np.float32) * 0.05,
        "b_inv2": rng.standard_normal((32,), dtype=np.float32) * 0.05,
        "W_goal": rng.standard_normal((32, 300), dtype=np.float32) * 0.05,
        "b_goal": rng.standard_normal((32,), dtype=np.float32) * 0.05,
        "in_W": rng.standard_normal((12, 32, 32), dtype=np.float32) * 0.05,
        "in_b": rng.standard_normal((12, 32), dtype=np.float32) * 0.05,
        "out_W": rng.standard_normal((12, 32, 32), dtype=np.float32) * 0.05,
        "out_b": rng.standard_normal((12, 32), dtype=np.float32) * 0.05,
        "att_W": rng.standard_normal((12, 32), dtype=np.float32) * 0.05,
        "att_b": rng.standard_normal((12,), dtype=np.float32) * 0.05,
    }
    out = kernel(**fake)
    print("kernel ran, out", out.shape, out.dtype, np.abs(out).max())



# revision 3
# speedup vs baseline: 1.1221x; 1.1221x over previous
"""Trainium2 Bass kernel for nn_AllObsPredictAtten (moe_routing).

Data-parallel over 8 NeuronCores: batch 8192 -> 1024 rows/core.
Per core, activations are kept feature-major ("transposed" layout:
[features on partitions, batch on free]) so every contraction runs on the
PE array with weights stationary.

 - x is DMA-loaded with an SWDGE f32->bf16 cast, transposed on the PE
   (bf16, 1 cyc/row), layer-1 matmuls run in bf16 with col-tiling
   (4 grid cells concurrently in the 128x128 array) and a block-diagonal
   one-hot matmul (4 cells per M=128 matmul).
 - Downstream layers (grid_comb, inv2, modules, attention) run in
   float32r (1 cyc/row at N=512, ~1e-4 accuracy).
 - All weight transforms (transposes, chunking, block-diagonals, bias
   stacking) are precomputed on the host and shipped as extra DRAM
   inputs (<1 MB, replicated to all cores).
 - softmax normalization is folded into the selection weights before the
   output-layer matmuls, so the final PSUM holds the finished output.

kernel(**inputs) caches the compiled 8-core program across calls.
"""
import sys

sys.path.insert(0, "/opt/trn_rl_repo")

import numpy as np
import ml_dtypes

import concourse.bacc as bacc
import concourse.bass as bass
import concourse.tile as tile
from concourse import mybir, bass2jax

F32 = mybir.dt.float32
F32R = mybir.dt.float32r
BF16 = mybir.dt.bfloat16

P = 128
BL = 1024           # batch rows per core
NCORES = 8
NSUP = BL // 512    # supertiles per core (N=512 each)
HID = 32

# x column map
GRID0 = 0            # 25 cells x 300
OH0 = 7500           # 25 cells x 7
GOAL0 = 7675         # 300
INV0 = 7975          # 10 cells x 300
XW = 10975

_CACHE = {}


# ----------------------------------------------------------------------------
# host-side parameter prep
# ----------------------------------------------------------------------------

def _prep_params(i):
    bf = ml_dtypes.bfloat16
    f32 = np.float32
    p = {}
    p["ident_bf"] = np.eye(P, dtype=bf)
    p["ident_f32"] = np.eye(P, dtype=f32)

    def chunkT(W, dt):  # W [32, F] -> [128, nk, 32] transposed chunks
        F = W.shape[1]
        nk = (F + 127) // 128
        out = np.zeros((P, nk, 32), dtype=dt)
        for k in range(nk):
            sz = min(128, F - 128 * k)
            out[:sz, k, :] = W[:, 128 * k:128 * k + sz].T.astype(dt)
        return out

    p["wg"] = chunkT(i["W_embed"], bf)        # [128, 3, 32] bf16
    p["winv1"] = chunkT(i["W_inv1"], bf)
    p["wgoal"] = chunkT(i["W_goal"], bf)
    p["wcomb"] = chunkT(i["W_comb"], f32)     # [128, 7, 32] f32r
    p["winv2"] = chunkT(i["W_inv2"], f32)     # [128, 3, 32]

    # block-diag onehot weights: quad q (cells 4q..4q+3, q6 = cell 24)
    # rows = A-local (7c, c<16) for q<=3, B-local (7c-112) for q>=4
    woh = np.zeros((P, 7, P), dtype=bf)
    WohT = i["W_onehot"].T  # [7, 32]
    for q in range(7):
        cells = range(4 * q, min(4 * q + 4, 25))
        for ci, c in enumerate(cells):
            r = 7 * c if c < 16 else 7 * c - 112
            woh[r:r + 7, q, 32 * ci:32 * ci + 32] = WohT.astype(bf)
    p["woh_bd"] = woh

    # in-layer grouped: [32i, 3j, 128(4a x 32o)]
    win = np.zeros((32, 3, P), dtype=f32)
    for j in range(3):
        for a in range(4):
            win[:, j, 32 * a:32 * a + 32] = i["in_W"][4 * j + a].T
    p["w_in_grp"] = win

    # out-layer stacked big-K: [128, 3, 32]: rows 32a+i = out_W[4j+a, o, i]
    wout = np.zeros((P, 3, 32), dtype=f32)
    for j in range(3):
        for a in range(4):
            wout[32 * a:32 * a + 32, j, :] = i["out_W"][4 * j + a].T
    p["w_out_big"] = wout

    p["w_att"] = i["att_W"].T.astype(f32).copy()      # [32, 12]
    E3 = np.zeros((12, 3, P), dtype=f32)
    for j in range(3):
        for a in range(4):
            E3[4 * j + a, j, 32 * a:32 * a + 32] = 1.0
    p["E3"] = E3
    p["ones12"] = np.ones((12, 1), dtype=f32)
    p["ones112"] = np.ones((1, 12), dtype=f32)
    p["outb"] = i["out_b"].astype(f32).copy()          # [12, 32]

    p["b_c1x4"] = np.tile(i["b_embed"], 4)[:, None].astype(f32)
    p["b_ohx4"] = np.tile(i["b_onehot"], 4)[:, None].astype(f32)
    p["b_i1x4"] = np.tile(i["b_inv1"], 4)[:, None].astype(f32)
    p["b_comb"] = i["b_comb"][:, None].astype(f32).copy()
    p["b_inv2"] = i["b_inv2"][:, None].astype(f32).copy()
    p["b_goal"] = i["b_goal"][:, None].astype(f32).copy()
    p["in_b_stk"] = i["in_b"].reshape(3, 128).T.astype(f32).copy()  # [128, 3]
    p["att_b"] = i["att_b"][:, None].astype(f32).copy()
    return p


_PARAM_DECLS = {
    "ident_bf": ([P, P], BF16),
    "ident_f32": ([P, P], F32),
    "wg": ([P, 3, 32], BF16),
    "winv1": ([P, 3, 32], BF16),
    "wgoal": ([P, 3, 32], BF16),
    "wcomb": ([P, 7, 32], F32R),
    "winv2": ([P, 3, 32], F32R),
    "woh_bd": ([P, 7, P], BF16),
    "w_in_grp": ([32, 3, P], F32R),
    "w_out_big": ([P, 3, 32], F32R),
    "w_att": ([32, 12], F32R),
    "E3": ([12, 3, P], F32R),
    "ones12": ([12, 1], F32R),
    "ones112": ([1, 12], F32R),
    "outb": ([12, 32], F32R),
    "b_c1x4": ([P, 1], F32),
    "b_ohx4": ([P, 1], F32),
    "b_i1x4": ([P, 1], F32),
    "b_comb": ([32, 1], F32),
    "b_inv2": ([32, 1], F32),
    "b_goal": ([32, 1], F32),
    "in_b_stk": ([P, 3], F32),
    "att_b": ([12, 1], F32),
}


# ----------------------------------------------------------------------------
# device program
# ----------------------------------------------------------------------------

def _build_nc(bench_r=0):
    import os
    DMA_ONLY = int(os.environ.get("DMA_ONLY", "0"))
    if DMA_ONLY:
        return _build_nc_dma_only(bench_r, DMA_ONLY)
    KVAR = int(os.environ.get("KVAR", "4"))
    T_BUFS = int(os.environ.get("T_BUFS", "3"))
    C1_BUFS = int(os.environ.get("C1_BUFS", "1"))
    IO_BUFS = int(os.environ.get("IO_BUFS", "2"))
    WK_BUFS = int(os.environ.get("WK_BUFS", "2"))
    EV_PAT = os.environ.get("EV_PAT", "ad")  # rotation: a=ACT d=DVE
    nc = bacc.Bacc(None, target_bir_lowering=False)
    X = nc.declare_dram_parameter("x_s", [BL, XW], F32, isOutput=False)
    H = nc.declare_dram_parameter("h_s", [BL, HID], F32, isOutput=False)
    prm = {}
    for name, (shape, dt) in _PARAM_DECLS.items():
        prm[name] = nc.declare_dram_parameter(name, shape, dt, isOutput=False)
    OUT = nc.declare_dram_parameter("out", [BL, HID], F32, isOutput=True)

    with tile.TileContext(nc) as tc:
        with (
            tc.tile_pool(name="const", bufs=1) as cp,
            tc.tile_pool(name="io", bufs=IO_BUFS) as io,
            tc.tile_pool(name="work", bufs=WK_BUFS) as wk,
            tc.tile_pool(name="psT", bufs=T_BUFS, space="PSUM") as psT,
            tc.tile_pool(name="ps", bufs=1, space="PSUM") as ps,
            tc.tile_pool(name="psC", bufs=C1_BUFS, space="PSUM") as psC,
        ):
            # ---- constants to SBUF
            c = {}
            for name, (shape, dt) in _PARAM_DECLS.items():
                t = cp.tile(shape, dt, tag=name)
                nc.sync.dma_start(out=t[:], in_=prm[name].ap())
                c[name] = t

            evict_ctr = [0]
            import contextlib
            loop_ctx = tc.For_i(0, bench_r, 1) if bench_r > 1 else contextlib.nullcontext()


            def evict(out_ap, in_ap):
                # rotate PSUM->SBUF eviction engine per EV_PAT
                e = EV_PAT[evict_ctr[0] % len(EV_PAT)]
                if e == "a":
                    nc.scalar.copy(out_ap, in_ap)
                else:
                    nc.vector.tensor_copy(out_ap, in_ap)
                evict_ctr[0] += 1

          # bench_r>1 wraps the whole per-core program in a hardware loop
          # (same compute each iteration; results overwritten) for timing.
            with loop_ctx:
              for s in range(NSUP):
                R0 = 512 * s
                # ======== og (onehot+goal) + hiddens ========
                og = io.tile([P, 4, 475], BF16, tag="og")
                nc.gpsimd.dma_start(
                    out=og[:],
                    in_=X.ap()[R0:R0 + 512, OH0:OH0 + 475]
                    .rearrange("(bt p) f -> p bt f", p=P),
                )
                hid = io.tile([P, 4, HID], F32, tag="hid")
                nc.sync.dma_start(
                    out=hid[:],
                    in_=H.ap()[R0:R0 + 512, :]
                    .rearrange("(bt p) f -> p bt f", p=P),
                )

                ohTA = wk.tile([112, 512], BF16, tag="ohTA")
                ohTB = wk.tile([63, 512], BF16, tag="ohTB")
                goalT = wk.tile([P, 3, 512], BF16, tag="goalT")
                hidT = wk.tile([32, 512], F32R, tag="hidT")

                for bt in range(4):
                    cols = slice(128 * bt, 128 * bt + 128)
                    bA = psT.tile([P, 4, P], BF16, tag="T")
                    nc.tensor.transpose(bA[0:112, 0, :], og[:, bt, 0:112], c["ident_bf"][:])
                    nc.tensor.transpose(bA[0:63, 1, :], og[:, bt, 112:175], c["ident_bf"][:])
                    nc.tensor.transpose(bA[:, 2, :], og[:, bt, 175:303], c["ident_bf"][:])
                    nc.tensor.transpose(bA[:, 3, :], og[:, bt, 303:431], c["ident_bf"][:])
                    bB = psT.tile([P, 4, P], BF16, tag="T")
                    nc.tensor.transpose(bB[0:44, 0, :], og[:, bt, 431:475], c["ident_bf"][:])
                    bH = psT.tile([32, P], F32, tag="T")
                    nc.tensor.transpose(bH[:], hid[:, bt, :], c["ident_f32"][:])
                    evict(ohTA[:, cols], bA[0:112, 0, :])
                    evict(ohTB[:, cols], bA[0:63, 1, :])
                    evict(goalT[:, 0:2, cols], bA[:, 2:4, :])
                    evict(goalT[0:44, 2, cols], bB[0:44, 0, :])
                    nc.scalar.copy(hidT[:, cols], bH[:])

                # ---- goal embedding
                gps = ps.tile([32, 512], F32, tag="misc")
                for k, Kk in enumerate((128, 128, 44)):
                    nc.tensor.matmul(
                        gps[:], c["wgoal"][0:Kk, k, :], goalT[0:Kk, k, :],
                        start=(k == 0), stop=(k == 2),
                    )
                goal_emb = wk.tile([32, 512], F32R, tag="goal_emb")
                nc.scalar.activation(goal_emb[:], gps[:],
                                     mybir.ActivationFunctionType.Relu,
                                     bias=c["b_goal"][:])

                # ---- attention -> normalized selection
                aps = ps.tile([12, 512], F32, tag="misc")
                nc.tensor.matmul(aps[:], c["w_att"][:], hidT[:], start=True, stop=True)
                expsel_f = wk.tile([12, 512], F32, tag="expsel_f")
                nc.scalar.activation(expsel_f[:], aps[:],
                                     mybir.ActivationFunctionType.Exp,
                                     bias=c["att_b"][:])
                expsel_r = wk.tile([12, 512], F32R, tag="expsel_r")
                nc.scalar.copy(expsel_r[:], expsel_f[:])
                sps = ps.tile([1, 512], F32, tag="misc")
                nc.tensor.matmul(sps[:], c["ones12"][:], expsel_r[:], start=True, stop=True)
                recip = wk.tile([1, 512], F32, tag="recip")
                nc.vector.reciprocal(recip[:], sps[:])
                recip_r = wk.tile([1, 512], F32R, tag="recip_r")
                nc.scalar.copy(recip_r[:], recip[:])
                rps = ps.tile([12, 512], F32, tag="misc")
                nc.tensor.matmul(rps[:], c["ones112"][:], recip_r[:], start=True, stop=True)
                seln_f = wk.tile([12, 512], F32, tag="seln_f")
                nc.vector.tensor_mul(seln_f[:], expsel_f[:], rps[:])
                sel_n = wk.tile([12, 512], F32R, tag="sel_n")
                nc.scalar.copy(sel_n[:], seln_f[:])

                # ======== grid quads -> c1/c2 -> comb accumulation ========
                combps = ps.tile([32, 512], F32, tag="comb")
                for q in range(7 if KVAR >= 2 else 0):
                    ncell = 4 if q < 6 else 1
                    W = 300 * ncell
                    Wl = W + 84  # widened so 44-col transposes read 128 cols
                    xq = io.tile([P, 4, 1284], BF16, tag="xq")
                    nc.gpsimd.dma_start(
                        out=xq[:, :, 0:Wl],
                        in_=X.ap()[R0:R0 + 512, 1200 * q:1200 * q + Wl]
                        .rearrange("(bt p) f -> p bt f", p=P),
                    )
                    xT = wk.tile([P, 12, 512], BF16, tag="xT")
                    for bt in range(4):
                        cols = slice(128 * bt, 128 * bt + 128)
                        nslot = 3 * ncell
                        banks = []
                        for b in range((nslot + 3) // 4):
                            bk = psT.tile([P, 4, P], BF16, tag="T")
                            banks.append(bk)
                        for ci in range(ncell):
                            for k, off in enumerate((0, 128, 256)):
                                slot = 3 * ci + k
                                bk = banks[slot // 4]
                                nc.tensor.transpose(
                                    bk[:, slot % 4, :],
                                    xq[:, bt, 300 * ci + off:300 * ci + off + 128],
                                    c["ident_bf"][:],
                                )
                        for b, bk in enumerate(banks):
                            w = min(4, nslot - 4 * b)
                            evict(xT[:, 4 * b:4 * b + w, cols], bk[:, 0:w, :])
                    # c1: grid cell matmuls (col-tiled)
                    c1ps = psC.tile([P, 512], F32, tag="c1")
                    for ci in range(ncell):
                        for k, Kk in enumerate((128, 128, 44)):
                            nc.tensor.matmul(
                                c1ps[32 * ci:32 * ci + 32, :],
                                c["wg"][0:Kk, k, :],
                                xT[0:Kk, 3 * ci + k, :],
                                start=(k == 0), stop=(k == 2),
                                tile_position=(0, 32 * ci),
                            )
                    # c2: block-diag onehot matmul
                    c2ps = ps.tile([P, 512], F32, tag="c2")
                    Mq = 32 * ncell
                    if q <= 3:
                        nc.tensor.matmul(c2ps[0:Mq, :], c["woh_bd"][0:112, q, 0:Mq],
                                         ohTA[:, :], start=True, stop=True)
                    else:
                        nc.tensor.matmul(c2ps[0:Mq, :], c["woh_bd"][0:63, q, 0:Mq],
                                         ohTB[:, :], start=True, stop=True)
                    c1r = wk.tile([P, 512], F32R, tag="c1r")
                    c2r = wk.tile([P, 512], F32R, tag="c2r")
                    nc.scalar.activation(c1r[0:Mq, :], c1ps[0:Mq, :],
                                         mybir.ActivationFunctionType.Relu,
                                         bias=c["b_c1x4"][0:Mq, :])
                    nc.scalar.activation(c2r[0:Mq, :], c2ps[0:Mq, :],
                                         mybir.ActivationFunctionType.Relu,
                                         bias=c["b_ohx4"][0:Mq, :])
                    Kq = 128 if q < 6 else 32
                    nc.tensor.matmul(combps[:], c["wcomb"][0:Kq, q, :], c1r[0:Kq, :],
                                     start=(q == 0), stop=False)
                    nc.tensor.matmul(combps[:], c["wcomb"][0:Kq, q, :], c2r[0:Kq, :],
                                     start=False, stop=(q == 6))

                # ======== inventory quads -> inv2 accumulation ========
                inv2ps = ps.tile([32, 512], F32, tag="inv2")
                for iq in range(3 if KVAR >= 3 else 0):
                    ncell = (4, 4, 2)[iq]
                    W = 300 * ncell
                    xq = io.tile([P, 4, 1284], BF16, tag="xq")
                    nc.gpsimd.dma_start(
                        out=xq[:, :, 0:W + (84 if iq < 2 else 0)],
                        in_=X.ap()[R0:R0 + 512,
                                   INV0 + 1200 * iq:INV0 + 1200 * iq + W + (84 if iq < 2 else 0)]
                        .rearrange("(bt p) f -> p bt f", p=P),
                    )
                    if iq == 2:
                        # x ends here; pad the widened-transpose overread with
                        # arbitrary (unused) data so all PSUM partitions get written
                        nc.gpsimd.dma_start(
                            out=xq[:, :, W:W + 84],
                            in_=X.ap()[R0:R0 + 512, 0:84]
                            .rearrange("(bt p) f -> p bt f", p=P),
                        )
                    xT = wk.tile([P, 12, 512], BF16, tag="xT")
                    for bt in range(4):
                        cols = slice(128 * bt, 128 * bt + 128)
                        nslot = 3 * ncell
                        banks = []
                        for b in range((nslot + 3) // 4):
                            bk = psT.tile([P, 4, P], BF16, tag="T")
                            banks.append(bk)
                        for ci in range(ncell):
                            for k, off in enumerate((0, 128, 256)):
                                slot = 3 * ci + k
                                bk = banks[slot // 4]
                                nc.tensor.transpose(
                                    bk[:, slot % 4, :],
                                    xq[:, bt, 300 * ci + off:300 * ci + off + 128],
                                    c["ident_bf"][:],
                                )
                        for b, bk in enumerate(banks):
                            w = min(4, nslot - 4 * b)
                            evict(xT[:, 4 * b:4 * b + w, cols], bk[:, 0:w, :])
                    i1ps = psC.tile([P, 512], F32, tag="c1")
                    for ci in range(ncell):
                        for k, Kk in enumerate((128, 128, 44)):
                            nc.tensor.matmul(
                                i1ps[32 * ci:32 * ci + 32, :],
                                c["winv1"][0:Kk, k, :],
                                xT[0:Kk, 3 * ci + k, :],
                                start=(k == 0), stop=(k == 2),
                                tile_position=(0, 32 * ci),
                            )
                    Mq = 32 * ncell
                    invr = wk.tile([P, 512], F32R, tag="invr")
                    nc.scalar.activation(invr[0:Mq, :], i1ps[0:Mq, :],
                                         mybir.ActivationFunctionType.Relu,
                                         bias=c["b_i1x4"][0:Mq, :])
                    Kiq = (128, 128, 64)[iq]
                    nc.tensor.matmul(inv2ps[:], c["winv2"][0:Kiq, iq, :], invr[0:Kiq, :],
                                     start=(iq == 0), stop=(iq == 2))

                # ======== net embeddings ========
                if KVAR < 2:
                    nc.tensor.matmul(combps[0:12, :], c["w_att"][:], hidT[:], start=True, stop=True)
                if KVAR < 3:
                    nc.tensor.matmul(inv2ps[0:12, :], c["w_att"][:], hidT[:], start=True, stop=True)
                grid_comb = wk.tile([32, 512], F32R, tag="grid_comb")
                nc.scalar.activation(grid_comb[:], combps[:],
                                     mybir.ActivationFunctionType.Relu,
                                     bias=c["b_comb"][:])
                inv_emb = wk.tile([32, 512], F32R, tag="inv_emb")
                nc.scalar.activation(inv_emb[:], inv2ps[:],
                                     mybir.ActivationFunctionType.Relu,
                                     bias=c["b_inv2"][:])

                # ======== modules + weighted output ========
                outps = ps.tile([32, 512], F32, tag="misc")
                srcs = (grid_comb, inv_emb, goal_emb)
                if KVAR < 4:
                    nc.tensor.matmul(outps[0:12, :], c["w_att"][:], hidT[:], start=True, stop=True)
                for j in range(3 if KVAR >= 4 else 0):
                    hps = psC.tile([P, 512], F32, tag="c1")
                    nc.tensor.matmul(hps[:], c["w_in_grp"][:, j, :], srcs[j][:],
                                     start=True, stop=True)
                    hj = wk.tile([P, 512], F32, tag="hj")
                    nc.scalar.activation(hj[:], hps[:],
                                         mybir.ActivationFunctionType.Tanh,
                                         bias=c["in_b_stk"][:, j:j + 1])
                    Bps = ps.tile([P, 512], F32, tag="c2")
                    nc.tensor.matmul(Bps[:], c["E3"][:, j, :], sel_n[:],
                                     start=True, stop=True)
                    gf = wk.tile([P, 512], F32, tag="gf")
                    nc.vector.tensor_mul(gf[:], hj[:], Bps[:])
                    gr = wk.tile([P, 512], F32R, tag="gr")
                    nc.scalar.copy(gr[:], gf[:])
                    nc.tensor.matmul(
                        outps[:], c["w_out_big"][:, j, :], gr[:],
                        start=(j == 0), stop=False,
                    )
                if KVAR >= 4:
                    nc.tensor.matmul(outps[:], c["outb"][:], sel_n[:],
                                     start=False, stop=True)

                # ======== transpose back + store ========
                out_sb = wk.tile([32, 512], F32, tag="out_sb")
                nc.scalar.copy(out_sb[:], outps[:])
                out_nat = io.tile([P, 4, HID], F32, tag="out_nat")
                for bt in range(4):
                    tf = psT.tile([P, HID], F32, tag="T")
                    nc.tensor.transpose(tf[:], out_sb[:, 128 * bt:128 * bt + 128],
                                        c["ident_f32"][0:32, 0:32])
                    nc.vector.tensor_copy(out_nat[:, bt, :], tf[:])
                nc.sync.dma_start(
                    out=OUT.ap()[R0:R0 + 512, :].rearrange("(bt p) f -> p bt f", p=P),
                    in_=out_nat[:],
                )

    nc.finalize()
    return nc


def _build_nc_dma_only(bench_r, mode):
    """DMA floor probes. mode 1: baseline load pattern. mode 2: merged exact
    loads (no overread). mode 3: merged f32 loads via HWDGE (no cast)."""
    import contextlib
    nc = bacc.Bacc(None, target_bir_lowering=False)
    X = nc.declare_dram_parameter("x_s", [BL, XW], F32, isOutput=False)
    H = nc.declare_dram_parameter("h_s", [BL, HID], F32, isOutput=False)
    prm = {}
    for name, (shape, dt) in _PARAM_DECLS.items():
        prm[name] = nc.declare_dram_parameter(name, shape, dt, isOutput=False)
    OUT = nc.declare_dram_parameter("out", [BL, HID], F32, isOutput=True)

    with tile.TileContext(nc) as tc:
        with (
            tc.tile_pool(name="io", bufs=2) as io,
        ):
            loop_ctx = tc.For_i(0, bench_r, 1) if bench_r > 1 else contextlib.nullcontext()
            with loop_ctx:
              for s in range(NSUP):
                R0 = 512 * s
                hid = io.tile([P, 4, HID], F32, tag="hid")
                nc.sync.dma_start(
                    out=hid[:],
                    in_=H.ap()[R0:R0 + 512, :]
                    .rearrange("(bt p) f -> p bt f", p=P),
                )
                if mode == 1:
                    og = io.tile([P, 4, 475], BF16, tag="og")
                    nc.gpsimd.dma_start(
                        out=og[:],
                        in_=X.ap()[R0:R0 + 512, OH0:OH0 + 475]
                        .rearrange("(bt p) f -> p bt f", p=P),
                    )
                    for q in range(7):
                        ncell = 4 if q < 6 else 1
                        Wl = 300 * ncell + 84
                        xq = io.tile([P, 4, 1284], BF16, tag="xq")
                        nc.gpsimd.dma_start(
                            out=xq[:, :, 0:Wl],
                            in_=X.ap()[R0:R0 + 512, 1200 * q:1200 * q + Wl]
                            .rearrange("(bt p) f -> p bt f", p=P),
                        )
                    for iq in range(3):
                        ncell = (4, 4, 2)[iq]
                        W = 300 * ncell
                        Wl = W + (84 if iq < 2 else 0)
                        xq = io.tile([P, 4, 1284], BF16, tag="xq")
                        nc.gpsimd.dma_start(
                            out=xq[:, :, 0:Wl],
                            in_=X.ap()[R0:R0 + 512,
                                       INV0 + 1200 * iq:INV0 + 1200 * iq + Wl]
                            .rearrange("(bt p) f -> p bt f", p=P),
                        )
                        if iq == 2:
                            nc.gpsimd.dma_start(
                                out=xq[:, :, W:W + 84],
                                in_=X.ap()[R0:R0 + 512, 0:84]
                                .rearrange("(bt p) f -> p bt f", p=P),
                            )
                elif mode == 2:
                    for c0, c1x in ((0, 3680), (3680, 7360), (7360, XW)):
                        Wd = c1x - c0
                        xq = io.tile([P, 4, 3680], BF16, tag="xq")
                        nc.gpsimd.dma_start(
                            out=xq[:, :, 0:Wd],
                            in_=X.ap()[R0:R0 + 512, c0:c1x]
                            .rearrange("(bt p) f -> p bt f", p=P),
                        )
                elif mode == 3:
                    for c0, c1x in ((0, 3680), (3680, 7360), (7360, XW)):
                        Wd = c1x - c0
                        xq = io.tile([P, 4, 3680], F32, tag="xq")
                        nc.sync.dma_start(
                            out=xq[:, :, 0:Wd],
                            in_=X.ap()[R0:R0 + 512, c0:c1x]
                            .rearrange("(bt p) f -> p bt f", p=P),
                        )
                out_nat = io.tile([P, 4, HID], F32, tag="out_nat")
                nc.vector.tensor_copy(out_nat[:], hid[:])
                nc.sync.dma_start(
                    out=OUT.ap()[R0:R0 + 512, :].rearrange("(bt p) f -> p bt f", p=P),
                    in_=out_nat[:],
                )

    nc.finalize()
    return nc


# ----------------------------------------------------------------------------
# 8-core runner (jit once, reuse)
# ----------------------------------------------------------------------------

def _make_runner(nc):
    import jax
    from jax.sharding import Mesh, PartitionSpec
    from jax.experimental.shard_map import shard_map

    bass2jax.install_neuronx_cc_hook()
    partition_name = nc.partition_id_tensor.name if nc.partition_id_tensor else None
    in_names, out_names, out_avals = [], [], []
    for alloc in nc.m.functions[0].allocations:
        if not isinstance(alloc, mybir.MemoryLocationSet):
            continue
        name = alloc.memorylocations[0].name
        if alloc.kind == "ExternalInput":
            if name != partition_name:
                in_names.append(name)
        elif alloc.kind == "ExternalOutput":
            out_names.append(name)
            out_avals.append(jax.core.ShapedArray(
                tuple(alloc.tensor_shape), mybir.dt.np(alloc.dtype)))
    n_params = len(in_names)
    n_outs = len(out_avals)
    in_names_full = in_names + out_names
    if partition_name is not None:
        in_names_full = in_names_full + [partition_name]
    donate = tuple(range(n_params, n_params + n_outs))

    def _body(*args):
        operands = list(args)
        if partition_name is not None:
            operands.append(bass2jax.partition_id_tensor())
        outs = bass2jax._bass_exec_p.bind(
            *operands,
            out_avals=tuple(out_avals),
            in_names=tuple(in_names_full),
            out_names=tuple(out_names),
            lowering_input_output_aliases=(),
            sim_require_finite=True,
            sim_require_nnan=True,
            nc=nc,
        )
        return tuple(outs)

    devices = jax.devices()[:NCORES]
    mesh = Mesh(np.asarray(devices), ("core",))
    in_specs = (PartitionSpec("core"),) * (n_params + n_outs)
    out_specs = (PartitionSpec("core"),) * n_outs
    sharded = jax.jit(
        shard_map(_body, mesh=mesh, in_specs=in_specs, out_specs=out_specs,
                  check_rep=False),
        donate_argnums=donate, keep_unused=True,
    )

    _CACHE["sharded"] = sharded
    _CACHE["body"] = _body
    _CACHE["mesh"] = mesh
    _CACHE["in_names"] = in_names
    _CACHE["out_names"] = out_names
    _CACHE["out_avals"] = out_avals
    _CACHE["n_params"] = n_params

    def run(global_ins):
        # global_ins: dict name -> np array with leading dim NCORES*per_core
        ins = [global_ins[name] for name in in_names]
        zeros = [np.zeros((NCORES * a.shape[0], *a.shape[1:]), a.dtype)
                 for a in out_avals]
        outs = sharded(*ins, *zeros)
        import jax as _j
        _j.block_until_ready(outs)
        return {name: np.asarray(outs[i]) for i, name in enumerate(out_names)}

    return run


def _get_runner():
    if "runner" not in _CACHE:
        nc = _build_nc()
        _CACHE["runner"] = _make_runner(nc)
    return _CACHE["runner"]


def kernel(**inputs):
    run = _get_runner()
    prm = _prep_params(inputs)
    global_ins = {
        "x_s": np.ascontiguousarray(inputs["x"], dtype=np.float32),
        "h_s": np.ascontiguousarray(inputs["hiddens"], dtype=np.float32),
    }
    for name in _PARAM_DECLS:
        a = prm[name]
        global_ins[name] = np.concatenate([a] * NCORES, axis=0)
    outs = run(global_ins)
    return outs["out"]  # [8192, 32] f32


if __name__ == "__main__":
    rng = np.random.default_rng(0)
    fake = {
        "x": rng.standard_normal((8192, XW), dtype=np.float32),
        "hiddens": rng.standard_normal((8192, HID), dtype=np.float32),
        "W_embed": rng.standard_normal((32, 300), dtype=np.float32) * 0.05,
        "b_embed": rng.standard_normal((32,), dtype=np.float32) * 0.05,
        "W_onehot": rng.standard_normal((32, 7), dtype=np.float32) * 0.05,
        "b_onehot": rng.standard_normal((32,), dtype=np.float32) * 0.05,
        "W_comb": rng.standard_normal((32, 800), dtype=np.float32) * 0.05,
        "b_comb": rng.standard_normal((32,), dtype=np.float32) * 0.05,
        "W_inv1": rng.standard_normal((32, 300), dtype=np.float32) * 0.05,
        "b_inv1": rng.standard_normal((32,), dtype=np.float32) * 0.05,
        "W_inv2": rng.standard_normal((32, 320), dtype=np.float32) * 0.05,
        "b_inv2": rng.standard_normal((32,), dtype=np.float32) * 0.05,
        "W_goal": rng.standard_normal((32, 300), dtype=np.float32) * 0.05,
        "b_goal": rng.standard_normal((32,), dtype=np.float32) * 0.05,
        "in_W": rng.standard_normal((12, 32, 32), dtype=np.float32) * 0.05,
        "in_b": rng.standard_normal((12, 32), dtype=np.float32) * 0.05,
        "out_W": rng.standard_normal((12, 32, 32), dtype=np.float32) * 0.05,
        "out_b": rng.standard_normal((12, 32), dtype=np.float32) * 0.05,
        "att_W": rng.standard_normal((12, 32), dtype=np.float32) * 0.05,
        "att_b": rng.standard_normal((12,), dtype=np.float32) * 0.05,
    }
    out = kernel(**fake)
    print("kernel ran, out", out.shape, out.dtype, np.abs(out).max())



# revision 4
# speedup vs baseline: 1.1406x; 1.0165x over previous
"""Trainium2 Bass kernel for nn_AllObsPredictAtten (moe_routing).

Data-parallel over 8 NeuronCores: batch 8192 -> 1024 rows/core.
Differences vs v1:
 - Exact merged SWDGE loads (no 84-col overread; 6 big casts + 1 HWDGE per
   512-row supertile instead of 12 calls with ~8% HBM overread).
 - Remainder (44-col) transposes are widened *within already-staged SBUF
   data* instead of widening the HBM read; only the last cell per load
   group does a true short transpose.
 - PSUM transpose banks hold 8 slots (full 2 KB bank); evictions move
   [128, 8*128] bf16 at once, on ACT via an f32-bitcast view (2 bf16 per
   f32 lane-element -> 2x ACT throughput) or on DVE as bf16 (2x mode).
 - Eviction engine chosen greedily by accumulated estimated busy-ns so
   ACT and DVE stay balanced.
"""
import sys

sys.path.insert(0, "/opt/trn_rl_repo")

import numpy as np
import ml_dtypes

import concourse.bacc as bacc
import concourse.bass as bass
import concourse.tile as tile
from concourse import mybir, bass2jax

F32 = mybir.dt.float32
F32R = mybir.dt.float32r
BF16 = mybir.dt.bfloat16

P = 128
BL = 1024           # batch rows per core
NCORES = 8
NSUP = BL // 512    # supertiles per core (N=512 each)
HID = 32

# x column map
GRID0 = 0            # 25 cells x 300
OH0 = 7500           # 25 cells x 7
GOAL0 = 7675         # 300
INV0 = 7975          # 10 cells x 300
XW = 10975

_CACHE = {}


# ----------------------------------------------------------------------------
# host-side parameter prep (identical to v1)
# ----------------------------------------------------------------------------

def _prep_params(i):
    import os
    BF16_DS_HOST = int(os.environ.get("BF16_DS", "0"))
    bf = ml_dtypes.bfloat16
    f32 = np.float32
    p = {}
    p["ident_bf"] = np.eye(P, dtype=bf)
    p["ident_f32"] = np.eye(P, dtype=f32)

    def chunkT(W, dt):  # W [32, F] -> [128, nk, 32] transposed chunks
        F = W.shape[1]
        nk = (F + 127) // 128
        out = np.zeros((P, nk, 32), dtype=dt)
        for k in range(nk):
            sz = min(128, F - 128 * k)
            out[:sz, k, :] = W[:, 128 * k:128 * k + sz].T.astype(dt)
        return out

    p["wg"] = chunkT(i["W_embed"], bf)        # [128, 3, 32] bf16
    p["winv1"] = chunkT(i["W_inv1"], bf)
    p["wgoal"] = chunkT(i["W_goal"], bf)
    ds = bf if BF16_DS_HOST else f32
    p["wcomb"] = chunkT(i["W_comb"], ds)      # [128, 7, 32]
    p["winv2"] = chunkT(i["W_inv2"], ds)      # [128, 3, 32]

    # block-diag onehot weights: quad q (cells 4q..4q+3, q6 = cell 24)
    # rows = A-local (7c, c<16) for q<=3, B-local (7c-112) for q>=4
    woh = np.zeros((P, 7, P), dtype=bf)
    WohT = i["W_onehot"].T  # [7, 32]
    for q in range(7):
        cells = range(4 * q, min(4 * q + 4, 25))
        for ci, c in enumerate(cells):
            r = 7 * c if c < 16 else 7 * c - 112
            woh[r:r + 7, q, 32 * ci:32 * ci + 32] = WohT.astype(bf)
    p["woh_bd"] = woh

    # in-layer grouped: [32i, 3j, 128(4a x 32o)]
    win = np.zeros((32, 3, P), dtype=ds)
    for j in range(3):
        for a in range(4):
            win[:, j, 32 * a:32 * a + 32] = i["in_W"][4 * j + a].T
    p["w_in_grp"] = win

    # out-layer stacked big-K: [128, 3, 32]: rows 32a+i = out_W[4j+a, o, i]
    wout = np.zeros((P, 3, 32), dtype=ds)
    for j in range(3):
        for a in range(4):
            wout[32 * a:32 * a + 32, j, :] = i["out_W"][4 * j + a].T
    p["w_out_big"] = wout

    p["w_att"] = i["att_W"].T.astype(f32).copy()      # [32, 12]
    E3 = np.zeros((12, 3, P), dtype=ds)
    for j in range(3):
        for a in range(4):
            E3[4 * j + a, j, 32 * a:32 * a + 32] = 1.0
    p["E3"] = E3
    p["ones12"] = np.ones((12, 1), dtype=f32)
    p["ones112"] = np.ones((1, 12), dtype=f32)
    p["outb"] = i["out_b"].astype(ds).copy()           # [12, 32]

    p["b_c1x4"] = np.tile(i["b_embed"], 4)[:, None].astype(f32)
    p["b_ohx4"] = np.tile(i["b_onehot"], 4)[:, None].astype(f32)
    p["b_i1x4"] = np.tile(i["b_inv1"], 4)[:, None].astype(f32)
    p["b_comb"] = i["b_comb"][:, None].astype(f32).copy()
    p["b_inv2"] = i["b_inv2"][:, None].astype(f32).copy()
    p["b_goal"] = i["b_goal"][:, None].astype(f32).copy()
    p["in_b_stk"] = i["in_b"].reshape(3, 128).T.astype(f32).copy()  # [128, 3]
    p["att_b"] = i["att_b"][:, None].astype(f32).copy()
    return p


import os as _os
_DS = BF16 if int(_os.environ.get("BF16_DS", "0")) else F32R
_PARAM_DECLS = {
    "ident_bf": ([P, P], BF16),
    "ident_f32": ([P, P], F32),
    "wg": ([P, 3, 32], BF16),
    "winv1": ([P, 3, 32], BF16),
    "wgoal": ([P, 3, 32], BF16),
    "wcomb": ([P, 7, 32], _DS),
    "winv2": ([P, 3, 32], _DS),
    "woh_bd": ([P, 7, P], BF16),
    "w_in_grp": ([32, 3, P], _DS),
    "w_out_big": ([P, 3, 32], _DS),
    "w_att": ([32, 12], F32R),
    "E3": ([12, 3, P], _DS),
    "ones12": ([12, 1], F32R),
    "ones112": ([1, 12], F32R),
    "outb": ([12, 32], _DS),
    "b_c1x4": ([P, 1], F32),
    "b_ohx4": ([P, 1], F32),
    "b_i1x4": ([P, 1], F32),
    "b_comb": ([32, 1], F32),
    "b_inv2": ([32, 1], F32),
    "b_goal": ([32, 1], F32),
    "in_b_stk": ([P, 3], F32),
    "att_b": ([12, 1], F32),
}


# ----------------------------------------------------------------------------
# device program
# ----------------------------------------------------------------------------

def _build_nc(bench_r=0):
    import os
    T_BUFS = int(os.environ.get("T_BUFS", "3"))
    C1_BUFS = int(os.environ.get("C1_BUFS", "1"))
    IO_BUFS = int(os.environ.get("IO_BUFS", "3"))
    WK_BUFS = int(os.environ.get("WK_BUFS", "2"))
    EV_FORCE = os.environ.get("EV_FORCE", "")  # "a"/"d" to force engine
    COMPUTE_ONLY = int(os.environ.get("COMPUTE_ONLY", "0"))
    T2 = int(os.environ.get("T2", "0"))
    BF16_DS = int(os.environ.get("BF16_DS", "0"))
    DRAIN_K = int(os.environ.get("DRAIN_K", "1"))
    nc = bacc.Bacc(None, target_bir_lowering=False)
    X = nc.declare_dram_parameter("x_s", [BL, XW], F32, isOutput=False)
    H = nc.declare_dram_parameter("h_s", [BL, HID], F32, isOutput=False)
    prm = {}
    for name, (shape, dt) in _PARAM_DECLS.items():
        prm[name] = nc.declare_dram_parameter(name, shape, dt, isOutput=False)
    OUT = nc.declare_dram_parameter("out", [BL, HID], F32, isOutput=True)

    with tile.TileContext(nc) as tc:
        with (
            tc.tile_pool(name="const", bufs=1) as cp,
            tc.tile_pool(name="io", bufs=IO_BUFS) as io,
            tc.tile_pool(name="wk", bufs=WK_BUFS) as wk,
            tc.tile_pool(name="wk1", bufs=1) as wk1,
            tc.tile_pool(name="psT", bufs=T_BUFS, space="PSUM") as psT,
            tc.tile_pool(name="ps", bufs=1, space="PSUM") as ps,
            tc.tile_pool(name="psC", bufs=C1_BUFS, space="PSUM") as psC,
        ):
            # ---- constants to SBUF
            c = {}
            for name, (shape, dt) in _PARAM_DECLS.items():
                t = cp.tile(shape, dt, tag=name)
                nc.sync.dma_start(out=t[:], in_=prm[name].ap())
                c[name] = t

            xconst = None
            if COMPUTE_ONLY:
                xconst = cp.tile([P, 4, 2400], BF16, tag="xconst")
                nc.gpsimd.memset(xconst[:], 0.125)

            # greedy engine balance for PSUM->SBUF evictions
            eng_ns = {"a": 0.0, "d": 0.0}

            def act_cost(f32_cols):
                return 150.0 + f32_cols / 1.2

            def dve_cost(cols, dt):
                rate = 1.92 if dt == BF16 else 0.96
                return 120.0 + cols / rate

            def track_act(f32_cols):
                eng_ns["a"] += act_cost(f32_cols)

            def track_dve(cols, dt=F32):
                eng_ns["d"] += dve_cost(cols, dt)

            def tpose(dst, in_, w):
                """transpose in_ [128, w] -> dst [0:w, 0:128] (bf16 psT slot);
                T2 splits into two row-halves on disjoint PE row groups so the
                second half's weight load overlaps the first half's stream."""
                if not T2:
                    nc.tensor.transpose(dst, in_, c["ident_bf"][:])
                else:
                    nc.tensor.transpose(dst[:, 0:64], in_.tensor_slice_p(0, 64) if False else in_[0:64, :],
                                        c["ident_bf"][0:64, 0:64])
                    nc.tensor.transpose(dst[:, 64:128], in_[64:128, :],
                                        c["ident_bf"][64:128, 64:128],
                                        tile_position=(64, 0))

            def evict(dst, src, cols, dt=BF16, bitcastable=True):
                """cols = free elems per lane in src dtype."""
                ca = act_cost(cols / 2 if (dt == BF16 and bitcastable) else cols)
                cd = dve_cost(cols, dt)
                pick = EV_FORCE or ("a" if eng_ns["a"] + ca <= eng_ns["d"] + cd
                                    else "d")
                if pick == "a":
                    eng_ns["a"] += ca
                    if dt == BF16 and bitcastable:
                        nc.scalar.copy(dst.bitcast(F32), src.bitcast(F32))
                    else:
                        nc.scalar.copy(dst, src)
                else:
                    eng_ns["d"] += cd
                    nc.vector.tensor_copy(dst, src)

            import contextlib
            loop_ctx = tc.For_i(0, bench_r, 1) if bench_r > 1 else contextlib.nullcontext()

            with loop_ctx:
              import collections
              for s in range(NSUP):
                R0 = 512 * s
                pending = collections.deque()

                def drain(k=1):
                    for _ in range(min(k, len(pending))):
                        pending.popleft()()

                # ======== loads (exact, merged) ========
                hid = io.tile([P, 4, HID], F32, tag="hid")
                nc.sync.dma_start(
                    out=hid[:],
                    in_=H.ap()[R0:R0 + 512, :]
                    .rearrange("(bt p) f -> p bt f", p=P),
                )
                if COMPUTE_ONLY:
                    xR = xconst
                else:
                    xR = io.tile([P, 4, 775], BF16, tag="xR")
                    nc.gpsimd.dma_start(
                        out=xR[:],
                        in_=X.ap()[R0:R0 + 512, 7200:7975]
                        .rearrange("(bt p) f -> p bt f", p=P),
                    )

                # ---- hidden transpose
                hidT = wk.tile([32, 512], F32R, tag="hidT")
                for bt in range(4):
                    cols = slice(128 * bt, 128 * bt + 128)
                    bH = psT.tile([32, P], F32, tag="T")
                    nc.tensor.transpose(bH[:], hid[:, bt, :], c["ident_f32"][:])
                    evict(hidT[:, cols], bH[:], 128, dt=F32)

                # ---- R transposes: slots 0-2 q6(cell24) k0/k1/rem(wide),
                #      3 ohA [0:112], 4 ohB [0:63], 5-6 goal k0/k1, 7 goal rem short
                xTR = wk.tile([P, 8, 512], BF16, tag="xTR")
                R_OFF = (0, 128, 256, 300, 412, 475, 603, 731)
                for bt in range(4):
                    cols = slice(128 * bt, 128 * bt + 128)
                    bk = psT.tile([P, 8, P], BF16, tag="T")
                    for j, off in enumerate(R_OFF):
                        w = 44 if j == 7 else 128
                        tpose(bk[0:w, j, :], xR[:, bt, off:off + w], w)
                    evict(xTR[:, 0:7, cols], bk[:, 0:7, :], 7 * 128)
                    evict(xTR[0:44, 7, cols], bk[0:44, 7, :], 128,
                          bitcastable=False)

                # ---- attention -> normalized selection (emitted early: its
                #      matmuls also keep the PE clock warm)
                aps = ps.tile([12, 512], F32, tag="misc")
                nc.tensor.matmul(aps[:], c["w_att"][:], hidT[:], start=True, stop=True)
                expsel_f = wk1.tile([12, 512], F32, tag="expsel_f")
                nc.scalar.activation(expsel_f[:], aps[:],
                                     mybir.ActivationFunctionType.Exp,
                                     bias=c["att_b"][:])
                track_act(512)
                expsel_r = wk1.tile([12, 512], F32R, tag="expsel_r")
                nc.scalar.copy(expsel_r[:], expsel_f[:])
                track_act(512)
                sps = ps.tile([1, 512], F32, tag="misc")
                nc.tensor.matmul(sps[:], c["ones12"][:], expsel_r[:], start=True, stop=True)
                recip = wk1.tile([1, 512], F32, tag="recip")
                nc.vector.reciprocal(recip[:], sps[:])
                track_dve(512)
                recip_r = wk1.tile([1, 512], F32R, tag="recip_r")
                nc.scalar.copy(recip_r[:], recip[:])
                track_act(512)
                rps = ps.tile([12, 512], F32, tag="misc")
                nc.tensor.matmul(rps[:], c["ones112"][:], recip_r[:], start=True, stop=True)
                seln_f = wk1.tile([12, 512], F32, tag="seln_f")
                nc.vector.tensor_mul(seln_f[:], expsel_f[:], rps[:])
                track_dve(512)
                sel_n = wk1.tile([12, 512], BF16 if BF16_DS else F32R, tag="sel_n")
                nc.scalar.copy(sel_n[:], seln_f[:])
                track_act(512)

                # ---- goal embedding (closure; drained between transposes)
                goal_emb = wk1.tile([32, 512], BF16 if BF16_DS else F32R, tag="goal_emb")

                def mk_goal():
                    def go():
                        gps = ps.tile([32, 512], F32, tag="misc")
                        for k, Kk in enumerate((128, 128, 44)):
                            nc.tensor.matmul(
                                gps[:], c["wgoal"][0:Kk, k, :], xTR[0:Kk, 5 + k, :],
                                start=(k == 0), stop=(k == 2),
                            )
                        nc.scalar.activation(goal_emb[:], gps[:],
                                             mybir.ActivationFunctionType.Relu,
                                             bias=c["b_goal"][:])
                        track_act(512)
                    return go
                pending.append(mk_goal())

                # ======== c1/c2/comb machinery ========
                combps = ps.tile([32, 512], F32, tag="comb")

                def push_quad(q, xT, slot0, ncell, c2_k):
                    """queue one quad: per-cell c1 closures, then c2+relu+comb."""
                    Mq = 32 * ncell
                    c1ps = psC.tile([P, 512], F32, tag="c1")
                    for ci in range(ncell):
                        def c1_cell(ci=ci, c1ps=c1ps, xT=xT, slot0=slot0, ncell=ncell):
                            for k, Kk in enumerate((128, 128, 44)):
                                nc.tensor.matmul(
                                    c1ps[32 * ci:32 * ci + 32, :],
                                    c["wg"][0:Kk, k, :],
                                    xT[0:Kk, slot0 + 3 * ci + k, :],
                                    start=(k == 0), stop=(k == 2),
                                    tile_position=(0, 32 * ci),
                                )
                        pending.append(c1_cell)

                    def c2_comb(q=q, c1ps=c1ps, Mq=Mq, c2_k=c2_k):
                        c2_src = (xTR[0:112, 3, :] if q <= 3 else xTR[0:63, 4, :])
                        c2ps = ps.tile([P, 512], F32, tag="c2")
                        nc.tensor.matmul(c2ps[0:Mq, :], c["woh_bd"][0:c2_k, q, 0:Mq],
                                         c2_src, start=True, stop=True)
                        DS = BF16 if BF16_DS else F32R
                        c1r = wk.tile([P, 512], DS, tag="c1r")
                        c2r = wk.tile([P, 512], DS, tag="c2r")
                        nc.scalar.activation(c1r[0:Mq, :], c1ps[0:Mq, :],
                                             mybir.ActivationFunctionType.Relu,
                                             bias=c["b_c1x4"][0:Mq, :])
                        track_act(512)
                        nc.scalar.activation(c2r[0:Mq, :], c2ps[0:Mq, :],
                                             mybir.ActivationFunctionType.Relu,
                                             bias=c["b_ohx4"][0:Mq, :])
                        track_act(512)
                        Kq = min(Mq, 128)
                        nc.tensor.matmul(combps[:], c["wcomb"][0:Kq, q, :], c1r[0:Kq, :],
                                         start=(q == 0), stop=False)
                        nc.tensor.matmul(combps[:], c["wcomb"][0:Kq, q, :], c2r[0:Kq, :],
                                         start=False, stop=(q == 6))
                    pending.append(c2_comb)

                inv2ps = ps.tile([32, 512], F32, tag="inv2")

                def push_inv_quad(iq, xT, slot0, ncell):
                    Mq = 32 * ncell
                    i1ps = psC.tile([P, 512], F32, tag="c1")
                    for ci in range(ncell):
                        def i1_cell(ci=ci, i1ps=i1ps, xT=xT, slot0=slot0):
                            for k, Kk in enumerate((128, 128, 44)):
                                nc.tensor.matmul(
                                    i1ps[32 * ci:32 * ci + 32, :],
                                    c["winv1"][0:Kk, k, :],
                                    xT[0:Kk, slot0 + 3 * ci + k, :],
                                    start=(k == 0), stop=(k == 2),
                                    tile_position=(0, 32 * ci),
                                )
                        pending.append(i1_cell)

                    def inv_tail(iq=iq, i1ps=i1ps, Mq=Mq):
                        invr = wk.tile([P, 512], BF16 if BF16_DS else F32R, tag="invr")
                        nc.scalar.activation(invr[0:Mq, :], i1ps[0:Mq, :],
                                             mybir.ActivationFunctionType.Relu,
                                             bias=c["b_i1x4"][0:Mq, :])
                        track_act(512)
                        Kiq = (128, 128, 64)[iq]
                        nc.tensor.matmul(inv2ps[:], c["winv2"][0:Kiq, iq, :],
                                         invr[0:Kiq, :],
                                         start=(iq == 0), stop=(iq == 2))
                    pending.append(inv_tail)

                def load_group(c0, width):
                    if COMPUTE_ONLY:
                        return xconst
                    xg = io.tile([P, 4, 2400], BF16, tag="xq")
                    nc.gpsimd.dma_start(
                        out=xg[:, :, 0:width],
                        in_=X.ap()[R0:R0 + 512, c0:c0 + width]
                        .rearrange("(bt p) f -> p bt f", p=P),
                    )
                    return xg

                def transpose_group(xg, ncell_g, width):
                    nslot = 3 * ncell_g
                    xT = wk.tile([P, 24, 512], BF16, tag="xT")
                    for bt in range(4):
                        cols = slice(128 * bt, 128 * bt + 128)
                        nbank = (nslot + 7) // 8
                        for b in range(nbank):
                            lo = 8 * b
                            hi = min(lo + 8, nslot)
                            bk = psT.tile([P, 8, P], BF16, tag="T")
                            short = None
                            for j in range(lo, hi):
                                cell, k = divmod(j, 3)
                                off = 300 * cell + 128 * k
                                w = min(128, width - off)
                                tpose(bk[0:w, j - lo, :],
                                      xg[:, bt, off:off + w], w)
                                if w < 128:
                                    short = (j - lo, w)
                            if short is None:
                                evict(xT[:, lo:hi, cols], bk[:, 0:hi - lo, :],
                                      (hi - lo) * 128)
                            else:
                                sj, w = short
                                if sj > 0:
                                    evict(xT[:, lo:lo + sj, cols],
                                          bk[:, 0:sj, :], sj * 128)
                                evict(xT[0:w, lo + sj, cols], bk[0:w, sj, :],
                                      128, bitcastable=False)
                            drain(DRAIN_K)
                    return xT

                # ======== grid groups ========
                for g in range(3):
                    xg = load_group(2400 * g, 2400)
                    xT = transpose_group(xg, 8, 2400)
                    for ql in range(2):
                        q = 2 * g + ql
                        push_quad(q, xT, 12 * ql, 4, 112 if q <= 3 else 63)

                # ======== inventory groups ========
                xga = load_group(INV0, 2400)
                xTa = transpose_group(xga, 8, 2400)
                push_inv_quad(0, xTa, 0, 4)
                push_inv_quad(1, xTa, 12, 4)

                if COMPUTE_ONLY:
                    xgb = xconst
                else:
                    xgb = io.tile([P, 4, 2400], BF16, tag="xq")
                    nc.gpsimd.dma_start(
                        out=xgb[:, :, 0:600],
                        in_=X.ap()[R0:R0 + 512, INV0 + 2400:INV0 + 3000]
                        .rearrange("(bt p) f -> p bt f", p=P),
                    )
                xTb = transpose_group(xgb, 2, 600)

                # q6 = cell 24, slots 0-2 of xTR
                push_quad(6, xTR, 0, 1, 63)
                push_inv_quad(2, xTb, 0, 2)

                drain(len(pending))

                grid_comb = wk1.tile([32, 512], BF16 if BF16_DS else F32R, tag="grid_comb")
                nc.scalar.activation(grid_comb[:], combps[:],
                                     mybir.ActivationFunctionType.Relu,
                                     bias=c["b_comb"][:])
                track_act(512)
                inv_emb = wk1.tile([32, 512], BF16 if BF16_DS else F32R, tag="inv_emb")
                nc.scalar.activation(inv_emb[:], inv2ps[:],
                                     mybir.ActivationFunctionType.Relu,
                                     bias=c["b_inv2"][:])
                track_act(512)

                # ======== modules + weighted output ========
                outps = ps.tile([32, 512], F32, tag="misc")
                srcs = (grid_comb, inv_emb, goal_emb)
                for j in range(3):
                    hps = psC.tile([P, 512], F32, tag="c1")
                    nc.tensor.matmul(hps[:], c["w_in_grp"][:, j, :], srcs[j][:],
                                     start=True, stop=True)
                    hj = wk.tile([P, 512], F32, tag="hj")
                    nc.scalar.activation(hj[:], hps[:],
                                         mybir.ActivationFunctionType.Tanh,
                                         bias=c["in_b_stk"][:, j:j + 1])
                    track_act(512)
                    Bps = ps.tile([P, 512], F32, tag="c2")
                    nc.tensor.matmul(Bps[:], c["E3"][:, j, :], sel_n[:],
                                     start=True, stop=True)
                    gf = wk.tile([P, 512], F32, tag="gf")
                    nc.vector.tensor_mul(gf[:], hj[:], Bps[:])
                    track_dve(512)
                    gr = wk.tile([P, 512], BF16 if BF16_DS else F32R, tag="gr")
                    nc.scalar.copy(gr[:], gf[:])
                    track_act(512)
                    nc.tensor.matmul(
                        outps[:], c["w_out_big"][:, j, :], gr[:],
                        start=(j == 0), stop=False,
                    )
                nc.tensor.matmul(outps[:], c["outb"][:], sel_n[:],
                                 start=False, stop=True)

                # ======== transpose back + store ========
                out_sb = wk1.tile([32, 512], F32, tag="out_sb")
                nc.scalar.copy(out_sb[:], outps[:])
                track_act(512)
                out_nat = io.tile([P, 4, HID], F32, tag="out_nat")
                for bt in range(4):
                    tf = psT.tile([P, HID], F32, tag="T")
                    nc.tensor.transpose(tf[:], out_sb[:, 128 * bt:128 * bt + 128],
                                        c["ident_f32"][0:32, 0:32])
                    nc.vector.tensor_copy(out_nat[:, bt, :], tf[:])
                    track_dve(32)
                nc.sync.dma_start(
                    out=OUT.ap()[R0:R0 + 512, :].rearrange("(bt p) f -> p bt f", p=P),
                    in_=out_nat[:],
                )

    nc.finalize()
    return nc


# ----------------------------------------------------------------------------
# 8-core runner (jit once, reuse)
# ----------------------------------------------------------------------------

def _make_runner(nc):
    import jax
    from jax.sharding import Mesh, PartitionSpec
    from jax.experimental.shard_map import shard_map

    bass2jax.install_neuronx_cc_hook()
    partition_name = nc.partition_id_tensor.name if nc.partition_id_tensor else None
    in_names, out_names, out_avals = [], [], []
    for alloc in nc.m.functions[0].allocations:
        if not isinstance(alloc, mybir.MemoryLocationSet):
            continue
        name = alloc.memorylocations[0].name
        if alloc.kind == "ExternalInput":
            if name != partition_name:
                in_names.append(name)
        elif alloc.kind == "ExternalOutput":
            out_names.append(name)
            out_avals.append(jax.core.ShapedArray(
                tuple(alloc.tensor_shape), mybir.dt.np(alloc.dtype)))
    n_params = len(in_names)
    n_outs = len(out_avals)
    in_names_full = in_names + out_names
    if partition_name is not None:
        in_names_full = in_names_full + [partition_name]
    donate = tuple(range(n_params, n_params + n_outs))

    def _body(*args):
        operands = list(args)
        if partition_name is not None:
            operands.append(bass2jax.partition_id_tensor())
        outs = bass2jax._bass_exec_p.bind(
            *operands,
            out_avals=tuple(out_avals),
            in_names=tuple(in_names_full),
            out_names=tuple(out_names),
            lowering_input_output_aliases=(),
            sim_require_finite=True,
            sim_require_nnan=True,
            nc=nc,
        )
        return tuple(outs)

    devices = jax.devices()[:NCORES]
    mesh = Mesh(np.asarray(devices), ("core",))
    in_specs = (PartitionSpec("core"),) * (n_params + n_outs)
    out_specs = (PartitionSpec("core"),) * n_outs
    sharded = jax.jit(
        shard_map(_body, mesh=mesh, in_specs=in_specs, out_specs=out_specs,
                  check_rep=False),
        donate_argnums=donate, keep_unused=True,
    )

    _CACHE["sharded"] = sharded
    _CACHE["body"] = _body
    _CACHE["mesh"] = mesh
    _CACHE["in_names"] = in_names
    _CACHE["out_names"] = out_names
    _CACHE["out_avals"] = out_avals
    _CACHE["n_params"] = n_params

    def run(global_ins):
        ins = [global_ins[name] for name in in_names]
        zeros = [np.zeros((NCORES * a.shape[0], *a.shape[1:]), a.dtype)
                 for a in out_avals]
        outs = sharded(*ins, *zeros)
        import jax as _j
        _j.block_until_ready(outs)
        return {name: np.asarray(outs[i]) for i, name in enumerate(out_names)}

    return run


def _get_runner():
    if "runner" not in _CACHE:
        nc = _build_nc()
        _CACHE["runner"] = _make_runner(nc)
    return _CACHE["runner"]


def kernel(**inputs):
    run = _get_runner()
    prm = _prep_params(inputs)
    global_ins = {
        "x_s": np.ascontiguousarray(inputs["x"], dtype=np.float32),
        "h_s": np.ascontiguousarray(inputs["hiddens"], dtype=np.float32),
    }
    for name in _PARAM_DECLS:
        a = prm[name]
        global_ins[name] = np.concatenate([a] * NCORES, axis=0)
    outs = run(global_ins)
    return outs["out"]  # [8192, 32] f32
